# revision 35
# baseline (speedup 1.0000x reference)
"""Multi-head attention (B=2,S=2048,E=1024,H=16,DK=DV=64) on 8 Trainium2 cores.

Sharding: core c handles batch c//4 and head-group c%4 (4 heads each).
Host sums the 4 partial outputs per batch and adds bo.

Design (vs the 281us baseline; all bf16 compute, fp32 psum):
 - The exp stream on the Scalar engine (128 x [128,1024] tiles, ~1.06us
   each = ~136us busy) is the floor; everything else is shaped to keep
   it fed and to minimize ramp + tail.
 - Scores: the two heads of a pair are row-tiled (K=64 each at PE rows
   0-63 / 64-127 via explicit tile_position) and write the two halves
   of ONE [128,1024] fp32 psum tile -> the pair runs concurrently
   (~320ns incl LDW) and one exp call covers both heads (q-half
   granularity, 512 queries x 128 keys x 2 heads per cycle).
 - ctx matmuls are emitted in groups (both q-halves + both heads
   adjacent, same-lhsT back-to-back) so the PE background weight
   buffer hides the LDWEIGHTS; flushed 2+ cycles behind the exp.
 - Mask multiply: one DVE op per cycle with a broadcast AP over both
   head-halves (bf16 2x mode, ~600-690ns).  fp8 anywhere in the
   e/mask path drops DVE to 1x and fp8 projections break the 2e-2
   max-rel budget, so everything stays bf16.
 - Norm per (head, block): den-row copy -> reciprocal_approx_fast
   (psum-sourced reciprocal silently returns garbage; must copy to
   SBUF first) -> gpsimd partition_broadcast -> DVE mul; norms are
   deferred into the next block's early cycles so the PE FIFO never
   blocks on them.  The last block runs qh-outer so its first q-half
   normalizes mid-block and the q0=1024 yproj mostly rides inside it.
 - y casts must be DVE tensor_copy: ScalarE copies truncate fp32->bf16
   and the bias correlates across cores (same rows), blowing max-err.
 - PSUM: st ring 2x2 banks (shared by scores/proj/yproj/keepalive
   tiles) + 2 ctx tiles 2x2 banks = 8.
 - Ramp: split weight DMA + minimal pre-block projections with evacs
   on the then-idle Scalar engine (Identity activation with AP bias).
 - Deferred q/k projection chunks and yproj tiles are spread across
   blocks 0-3 as cycle-inserts sized to each block's PE slack; dummy
   keepalive matmuls bridge block-boundary norm lulls so the PE HAM
   clock stays at 2.4GHz.
"""

import numpy as np
import ml_dtypes

import concourse.bacc as bacc
import concourse.mybir as mybir
import concourse.tile as tile
from concourse import bass_utils

BF = ml_dtypes.bfloat16
F8 = ml_dtypes.float8_e4m3
dt = mybir.dt

NCORES = 8
WSCALE = 1.0
_DEBUG = False


def _emit(nc, tc, inp, y_d, S, E, HL, DK):
    EC = E // 128          # contraction chunks for the projections
    NPAIR = EC // 2        # fp8 DoubleRow chunk pairs
    NT = S // 128          # seq tiles
    DKL = HL * DK          # local head dims (256)
    NP = DKL // 128        # q/k partition tiles (pairs of heads)
    Exp = mybir.ActivationFunctionType.Exp
    DR = mybir.MatmulPerfMode.DoubleRow
    CS = 512
    ESCALE = 0.125 / (WSCALE * WSCALE)
    NQH = S // CS          # 512-wide q-half ranges (4)

    persist = tc.alloc_tile_pool(name="persist", bufs=1)
    qT = [persist.tile([128, S], dt.bfloat16, name=f"qT{m}") for m in range(NP)]
    kT = [persist.tile([128, S], dt.bfloat16, name=f"kT{m}") for m in range(NP)]
    cT = [persist.tile([128, S], dt.bfloat16, name=f"cT{m}") for m in range(NP)]
    VW = HL * (DK + 1)
    vAall = persist.tile([128, NT * VW], dt.bfloat16, name="vAall")
    vA = [vAall[:, t * VW:(t + 1) * VW] for t in range(NT)]

    # bf16 weights: [wq chunks | wk chunks | wv chunks | wo tiles]
    WSEG = EC * DKL
    wall16 = persist.tile([128, 3 * WSEG + NP * E], dt.bfloat16, name="wall16")
    w_sb = {}
    for i, nm in enumerate(("wq", "wk", "wv")):
        w_sb[nm] = [wall16[:, i * WSEG + c * DKL:i * WSEG + (c + 1) * DKL]
                    for c in range(EC)]
    wv_sb = w_sb["wv"]
    wo_sb = [wall16[:, 3 * WSEG + p * E:3 * WSEG + (p + 1) * E]
             for p in range(NP)]
    nc.sync.dma_start(wall16[:, 0:2 * WSEG], inp["wall16"][:, 0:2 * WSEG])
    bqk = persist.tile([128, 2 * NP], dt.float32, name="bqk")
    nc.sync.dma_start(bqk[:], inp["bqk"][:])
    bv1 = persist.tile([1, DKL], dt.bfloat16, name="bv1")
    nc.sync.dma_start(bv1[:], inp["bv"][:])

    # preload the Exp activation-table while the input DMAs stream in
    dumm = persist.tile([1, 4], dt.float32, name="dumm")
    nc.gpsimd.memset(dumm[:], 0.0)
    dumo = persist.tile([1, 4], dt.float32, name="dumo")
    nc.scalar.activation(dumo[:], dumm[:], Exp)

    bvb = persist.tile([128, DKL], dt.bfloat16, name="bvb")
    nc.gpsimd.partition_broadcast(bvb[:], bv1[:])
    nc.gpsimd.memset(vAall[:], 1.0)

    # mask tiles [128, 512] per (qi, t); ring covers 2 blocks + prefetch
    mpool = tc.alloc_tile_pool(name="mask", bufs=1)
    mh = {}
    for qi in range(NQH):
        for t in range(NT):
            mh[(qi, t)] = mpool.tile([128, CS], dt.bfloat16, tag="m",
                                     bufs=32, name=f"mask{qi}_{t}")

    npool = tc.alloc_tile_pool(name="nrm", bufs=1)
    ypool = tc.alloc_tile_pool(name="ysb", bufs=2)
    epool = tc.alloc_tile_pool(name="es", bufs=1)
    ERING = 11

    # bf16 x tiles: one [128, EC*1024] tile per (q/k, seq-half) -> a
    # single DMA with 16KB contiguous per-partition lines
    x8pool = tc.alloc_tile_pool(name="x8", bufs=1)
    xh = {nm: [x8pool.tile([128, EC * (S // 2)], dt.bfloat16, tag="x8",
                           bufs=4, name=f"xh{nm}_{h}") for h in range(2)]
          for nm in ("q", "k")}
    xvpool = tc.alloc_tile_pool(name="xvp", bufs=1)
    xvt = [xvpool.tile([128, EC * 128], dt.bfloat16, tag="xv", bufs=10,
                       name=f"xvt{t}") for t in range(NT)]

    # ---- DMA schedule (issue order ~= completion order) -------------------
    def dma_x8(nm, h):
        nc.sync.dma_start(xh[nm][h][:], inp["x" + nm][h])

    dma_x8("q", 0)
    dma_x8("k", 0)
    for t in range(4):
        nc.sync.dma_start(mh[(0, t)][:], inp["mask"][:, t, 0:CS])
    dma_x8("k", 1)
    nc.sync.dma_start(wall16[:, 2 * WSEG:], inp["wall16"][:, 2 * WSEG:])
    for tv in range(NT):
        nc.sync.dma_start(xvt[tv][:], inp["xv"][tv])
        if tv < 12:
            for t in range(4 + 2 * tv, 4 + 2 * (tv + 1)):
                qi, tt = divmod(t, NT)
                if qi < 2:
                    nc.sync.dma_start(mh[(qi, tt)][:],
                                      inp["mask"][:, tt, qi * CS:(qi + 1) * CS])
    dma_x8("q", 1)
    for t in range(28, 2 * NT):
        qi, tt = divmod(t, NT)
        nc.sync.dma_start(mh[(qi, tt)][:],
                          inp["mask"][:, tt, qi * CS:(qi + 1) * CS])
    for qi in (2, 3):
        for tt in range(NT):
            nc.sync.dma_start(mh[(qi, tt)][:],
                              inp["mask"][:, tt, qi * CS:(qi + 1) * CS])

    stpool = tc.alloc_tile_pool(name="stps", bufs=1, space="PSUM")

    PROJ = {"q": (qT, 0), "k": (kT, NP)}
    _kan = [0]

    def _proj_group(nm, m, col, act_evac=False):
        # one 512-col range of the q or k projection (bf16)
        dst, boff = PROJ[nm]
        h, n0 = divmod(col, S // 2)
        ps = stpool.tile([128, CS], dt.float32, tag="st", bufs=2,
                         name=f"{nm}ps{m}_{col}")
        for c in range(EC):
            nc.tensor.matmul(ps[:], w_sb["w" + nm][c][:, 128 * m:128 * (m + 1)],
                             xh[nm][h][:, c * (S // 2) + n0:
                                        c * (S // 2) + n0 + CS],
                             start=(c == 0), stop=(c == EC - 1))
        if act_evac:
            # ramp-time evac on the (idle) Scalar engine
            nc.scalar.activation(dst[m][:, col:col + CS], ps[:],
                                 mybir.ActivationFunctionType.Identity,
                                 bias=bqk[:, boff + m:boff + m + 1])
        else:
            nc.vector.tensor_scalar_add(
                dst[m][:, col:col + CS], ps[:],
                bqk[:, boff + m:boff + m + 1])

    def _v_group(t):
        vps = stpool.tile([128, CS], dt.float32, tag="st", bufs=2,
                          name=f"vps{t}")
        for c in range(EC):
            nc.tensor.matmul(vps[:, 0:DKL],
                             xvt[t][:, c * 128:(c + 1) * 128],
                             wv_sb[c][:],
                             start=(c == 0), stop=(c == EC - 1))
        nc.vector.tensor_add(
            vA[t][:].rearrange("p (h c) -> p h c", h=HL)[:, :, 0:DK],
            vps[:, 0:DKL].rearrange("p (h c) -> p h c", h=HL),
            bvb[:].rearrange("p (h c) -> p h c", h=HL))

    def _keepalive(n):
        # dummy matmuls into a throwaway st-ring slot: keep the PE HAM-warm
        # across block-boundary norm lulls (idle >3.4us drops PE to 1.2GHz)
        ka = stpool.tile([128, CS], dt.float32, tag="st", bufs=2,
                         name=f"ka{_kan[0]}")
        _kan[0] += 1
        for i in range(n):
            nc.tensor.matmul(ka[:, 0:DKL], bvb[:, 0:128], bvb[:],
                             start=True, stop=True)

    def _yproj_group(s, tail=False):
        yp = stpool.tile([128, E], dt.float32, tag="st", bufs=2,
                         name=f"yp{s}")
        for p in range(NP):
            for e0 in range(0, E, CS):
                nc.tensor.matmul(yp[:, e0:e0 + CS],
                                 cT[p][:, s * 128:(s + 1) * 128],
                                 wo_sb[p][:, e0:e0 + CS],
                                 start=(p == 0), stop=(p == NP - 1))
        ysb = ypool.tile([128, E], dt.bfloat16, tag="y", bufs=2,
                         name=f"ysb{s}")
        nc.vector.tensor_copy(ysb[:], yp[:])
        nc.sync.dma_start(y_d[s * 128:(s + 1) * 128, :], ysb[:])

    def _norm(ctx_t, hp, h, q0, w=None):
        # normalize cT[hp] rows sub..sub+64: dn copy -> recip ->
        # gpsimd bcast -> mul.  w=None: full 1024 cols; w=0/1: one 512 half.
        sub = (h % 2) * 64
        wd = 2 * CS if w is None else CS
        col = q0 + (0 if w is None else w * CS)
        dn = npool.tile([1, 2 * CS], dt.float32, tag="dn",
                        name=f"dn{h}_{col}")
        nc.vector.tensor_copy(dn[:, 0:wd], ctx_t[DK:DK + 1,
                                                 (col - q0):(col - q0) + wd])
        rec32 = npool.tile([1, 2 * CS], dt.float32, tag="rec32",
                           name=f"rec32_{h}_{col}")
        nc.vector.reciprocal_approx_fast(rec32[:, 0:wd], dn[:, 0:wd])
        if _DEBUG and h == 0 and col == 0 and w is None:
            nc.sync.dma_start(inp["drec"][:], rec32[:])
            nc.sync.dma_start(inp["dden"][:], dn[:])
        bd = npool.tile([64, 2 * CS], dt.float32, tag="bd", bufs=2,
                        name=f"bd{h}_{col}")
        nc.gpsimd.partition_broadcast(bd[:, 0:wd], rec32[:, 0:wd], opt=False)
        nc.vector.tensor_mul(cT[hp][sub:sub + 64, col:col + wd],
                             ctx_t[0:DK, (col - q0):(col - q0) + wd],
                             bd[:, 0:wd])

    # ---- pre-block projections ---------------------------------------------
    _proj_group("q", 0, 0, act_evac=True)
    _proj_group("q", 0, CS, act_evac=True)
    _proj_group("k", 0, 0, act_evac=True)

    # ---- attention blocks --------------------------------------------------
    # blocks: (q0=0,hp0), (q0=0,hp1), (q0=1024,hp0), (q0=1024,hp1)
    # inserts are (cycle -> thunk) maps; cycles 0..31.  The last block runs
    # qh-outer so its first q-half can normalize + start yproj mid-block.
    with tc.tile_pool(name="ctxps", bufs=1, space="PSUM") as ctxpool:
        blocks = [(0, 0), (0, 1), (1024, 0), (1024, 1)]
        pending_norms = []
        for bi, (q0, hp) in enumerate(blocks):
            heads = (2 * hp, 2 * hp + 1)
            ctxs = {}
            pend = []
            inserts = {}

            def ins(cyc, fn):
                inserts.setdefault(cyc, []).append(fn)

            for i, fn in enumerate(pending_norms):
                ins(1 + 2 * i, fn)
            pending_norms = []

            if bi == 0:
                ins(0, lambda: _proj_group("k", 0, CS))
                for i in range(2):
                    ins(4 + 2 * i, lambda i=i: _proj_group("k", 0, 1024 + CS * i))
                for i in range(16):
                    ins(8 + i, lambda i=i: _v_group(i))
                ins(22, lambda: _proj_group("k", 1, 0))
                ins(24, lambda: _proj_group("k", 1, CS))
                ins(26, lambda: _proj_group("q", 1, 0))
                ins(28, lambda: _proj_group("q", 1, CS))
            elif bi == 1:
                for i in range(2):
                    ins(1 + 2 * i, lambda i=i: _proj_group("k", 1, 1024 + CS * i))
                for i in range(2):
                    ins(5 + 2 * i, lambda i=i: _proj_group("q", 0, 1024 + CS * i))
            elif bi == 2:
                for i in range(2):
                    ins(6 + 2 * i, lambda i=i: _proj_group("q", 1, 1024 + CS * i))
                for i in range(6):
                    ins(10 + 3 * i, lambda i=i: _yproj_group(i))
            elif bi == 3:
                # qh-outer: qa complete at cyc 15; norms + q0=0 leftovers +
                # q0=1024 first-half yproj ride the qb cycles
                ins(9, lambda: _yproj_group(6))
                ins(11, lambda: _yproj_group(7))
                ins(18, lambda: _norm(ctxs[heads[0]], hp, heads[0], q0, w=0))
                ins(20, lambda: _norm(ctxs[heads[1]], hp, heads[1], q0, w=0))
                for i in range(4):
                    ins(22 + 2 * i, lambda i=i: _yproj_group(8 + i))

            def _ctx_emit_group(entries):
                # grouped by head so same-lhsT matmuls are adjacent (the
                # PE background weight buffer then hides the LDWEIGHTS)
                for ci, h in enumerate(heads):
                    if h not in ctxs:
                        ctxs[h] = ctxpool.tile(
                            [128, 2 * CS], dt.float32,
                            tag=f"ctx{h % 2}", name=f"ctx{h}_{q0}")
                    for (t_, qh_, e_) in entries:
                        nc.tensor.matmul(
                            ctxs[h][0:DK + 1, qh_ * CS:(qh_ + 1) * CS],
                            vA[t_][:, h * (DK + 1):(h + 1) * (DK + 1)],
                            e_[:, ci * CS:(ci + 1) * CS],
                            start=(t_ == 0), stop=(t_ == NT - 1))
                for p in entries:
                    pend.remove(p)

            def _ctx_flush(upto, limit=None):
                ready = [p for p in pend if p[0] <= upto]
                if limit is not None:
                    ready = ready[:limit]
                if ready:
                    _ctx_emit_group(ready)

            if bi == 3:
                order = [(t, qh) for qh in range(2) for t in range(NT)]
            else:
                order = [(t, qh) for t in range(NT) for qh in range(2)]
            for cyc, (t, qh) in enumerate(order):
                qc = q0 + qh * CS
                qi = qc // CS
                st = stpool.tile([128, 2 * CS], dt.float32, tag="st",
                                 bufs=2, name=f"st{t}_{qh}_{bi}")
                for ci, h in enumerate(heads):
                    sub = (h % 2) * 64
                    nc.tensor.matmul(
                        st[:, ci * CS:(ci + 1) * CS],
                        kT[hp][sub:sub + DK, t * 128:(t + 1) * 128],
                        qT[hp][sub:sub + DK, qc:qc + CS],
                        start=True, stop=True,
                        tile_position=(sub, 0))
                e = epool.tile([128, 2 * CS], dt.bfloat16, tag="e",
                               bufs=ERING, name=f"e{t}_{qh}_{bi}")
                nc.scalar.activation(e[:], st[:], Exp, scale=ESCALE)
                nc.vector.tensor_mul(
                    e[:].rearrange("p (two n) -> p two n", two=2),
                    e[:].rearrange("p (two n) -> p two n", two=2),
                    mh[(qi, t)][:].unsqueeze(1).broadcast_to(
                        [128, 2, CS]))
                if _DEBUG and bi == 0 and t == 0 and qh == 0:
                    nc.sync.dma_start(inp["de0"][:], e[:])
                pend.append((t, qh, e))
                for fn in inserts.get(cyc, ()):
                    if fn is not None:
                        fn()
                if cyc % 2 == 0:
                    continue
                if bi == 0:
                    _ctx_flush(min(t - 1, cyc - 9), limit=6)
                elif bi == 3:
                    # keep a 2-cycle lag behind production order
                    grp = list(pend[:-2])[:6]
                    if grp:
                        _ctx_emit_group(grp)
                else:
                    _ctx_flush(t - 2, limit=6)
            _ctx_flush(NT - 1)
            if bi < 3:
                _keepalive(14)
            if bi == 3:
                _norm(ctxs[heads[0]], hp, heads[0], q0, w=1)
                _norm(ctxs[heads[1]], hp, heads[1], q0, w=1)
                _keepalive(24)
            else:
                pending_norms = [
                    (lambda h=h, c=ctxs, hp=hp, q0=q0: _norm(c[h], hp, h, q0))
                    for h in heads]
        # tail: last yproj quartet
        for s in range(12, 16):
            _yproj_group(s)

    if _DEBUG:
        for nm, tl in [("dqT0", qT[0]), ("dqT1", qT[1]), ("dkT0", kT[0]),
                       ("dkT1", kT[1]), ("dcT0", cT[0]), ("dcT1", cT[1])]:
            nc.sync.dma_start(inp[nm][:], tl[:])
        nc.sync.dma_start(inp["dvA"][:], vAall[:])

    xvpool.release()
    x8pool.release()
    epool.release()
    ypool.release()
    npool.release()
    stpool.release()
    mpool.release()
    persist.release()


def _build(S, E, HL, DK):
    EC = E // 128
    NPAIR = EC // 2
    NT = S // 128
    DKL = HL * DK
    NP = DKL // 128
    nc = bacc.Bacc("TRN2", target_bir_lowering=False, debug=False,
                   num_devices=NCORES)
    inp = {}
    for nm in ("xq", "xk"):
        inp[nm] = nc.dram_tensor(nm, [2, 128, EC * (S // 2)], dt.bfloat16,
                                 kind="ExternalInput").ap()
    inp["xv"] = nc.dram_tensor("xv", [NT, 128, EC * 128], dt.bfloat16,
                               kind="ExternalInput").ap()
    inp["wall16"] = nc.dram_tensor("wall16", [128, 3 * EC * DKL + NP * E],
                                   dt.bfloat16, kind="ExternalInput").ap()
    inp["bqk"] = nc.dram_tensor("bqk", [128, 2 * NP], dt.float32,
                                kind="ExternalInput").ap()
    inp["bv"] = nc.dram_tensor("bv", [1, DKL], dt.bfloat16,
                               kind="ExternalInput").ap()
    inp["mask"] = nc.dram_tensor("mask", [128, NT, S], dt.bfloat16,
                                 kind="ExternalInput").ap()
    if _DEBUG:
        for nm in ("dqT0", "dqT1", "dkT0", "dkT1", "dcT0", "dcT1"):
            inp[nm] = nc.dram_tensor(nm, [128, S], dt.bfloat16,
                                     kind="ExternalOutput").ap()
        inp["dvA"] = nc.dram_tensor("dvA", [128, NT * HL * (DK + 1)],
                                    dt.bfloat16, kind="ExternalOutput").ap()
        inp["de0"] = nc.dram_tensor("de0", [128, 1024], dt.bfloat16,
                                    kind="ExternalOutput").ap()
        inp["drec"] = nc.dram_tensor("drec", [1, 1024], dt.float32,
                                     kind="ExternalOutput").ap()
        inp["dden"] = nc.dram_tensor("dden", [1, 1024], dt.float32,
                                     kind="ExternalOutput").ap()
    y_d = nc.dram_tensor("y", [S, E], dt.bfloat16, kind="ExternalOutput").ap()

    with tile.TileContext(nc) as tc:
        _emit(nc, tc, inp, y_d, S, E, HL, DK)
    nc.compile()
    return nc


_CACHE = {}
_TRACE = False
_TRACE_CORES = (0,)
_LAST_RESULT = None


def _get_nc(S, E, HL, DK):
    key = (S, E, HL, DK)
    if key not in _CACHE:
        _CACHE[key] = _build(S, E, HL, DK)
    return _CACHE[key]


_RUNNER_CACHE = {}


def _get_runner(nc):
    """Cached variant of bass2jax.run_bass_via_pjrt's multi-core path: build
    the jitted shard_map executable once and reuse it across kernel() calls
    (a fresh jax.jit per call re-traces and may recompile)."""
    if id(nc) in _RUNNER_CACHE:
        return _RUNNER_CACHE[id(nc)]
    import jax
    import concourse.mybir as _mybir
    from concourse import bass2jax
    from jax.sharding import Mesh, PartitionSpec
    from jax.experimental.shard_map import shard_map

    bass2jax.install_neuronx_cc_hook()
    pid_name = nc.partition_id_tensor.name if nc.partition_id_tensor else None
    in_names, out_names, out_avals, zero_shapes = [], [], [], []
    for alloc in nc.m.functions[0].allocations:
        if not isinstance(alloc, _mybir.MemoryLocationSet):
            continue
        name = alloc.memorylocations[0].name
        if alloc.kind == "ExternalInput":
            if name != pid_name:
                in_names.append(name)
        elif alloc.kind == "ExternalOutput":
            out_names.append(name)
            shape = tuple(alloc.tensor_shape)
            dtype = _mybir.dt.np(alloc.dtype)
            out_avals.append(jax.core.ShapedArray(shape, dtype))
            zero_shapes.append((shape, dtype))
    n_params = len(in_names)
    n_outs = len(out_avals)
    all_names = in_names + out_names
    if pid_name is not None:
        all_names = all_names + [pid_name]

    def _body(*args):
        operands = list(args)
        if pid_name is not None:
            operands.append(bass2jax.partition_id_tensor())
        return tuple(bass2jax._bass_exec_p.bind(
            *operands,
            out_avals=tuple(out_avals),
            in_names=tuple(all_names),
            out_names=tuple(out_names),
            lowering_input_output_aliases=(),
            sim_require_finite=True,
            sim_require_nnan=True,
            nc=nc,
        ))

    devices = jax.devices()[:NCORES]
    mesh = Mesh(np.asarray(devices), ("core",))
    donate = tuple(range(n_params, n_params + n_outs))
    sharded = jax.jit(
        shard_map(_body, mesh=mesh,
                  in_specs=(PartitionSpec("core"),) * (n_params + n_outs),
                  out_specs=(PartitionSpec("core"),) * n_outs,
                  check_rep=False),
        donate_argnums=donate, keep_unused=True)

    def run(in_maps):
        concat_in = [np.concatenate([np.asarray(m[nm]) for m in in_maps], axis=0)
                     for nm in in_names]
        concat_zeros = [np.zeros((NCORES * s[0], *s[1:]), d)
                        for s, d in zero_shapes]
        outs = sharded(*concat_in, *concat_zeros)
        return [
            {nm: np.asarray(outs[i]).reshape(NCORES, *out_avals[i].shape)[c]
             for i, nm in enumerate(out_names)}
            for c in range(NCORES)
        ]

    _RUNNER_CACHE[id(nc)] = run
    return run


def run_sharded(query, key, value, mask, Wq, bq, Wk, bk, Wv, bv, Wo, bo):
    """Full-input -> full-output runner (generic shapes)."""
    global _LAST_RESULT
    query, key, value = (np.asarray(a, np.float32) for a in (query, key, value))
    mask = np.asarray(mask)
    Wq, bq, Wk, bk, Wv, bv, Wo, bo = (
        np.asarray(a, np.float32) for a in (Wq, bq, Wk, bk, Wv, bv, Wo, bo))

    B, S, E = query.shape
    HDK = Wq.shape[1]
    DKv = 64
    H = HDK // DKv
    GPB = NCORES // B                 # cores per batch
    HL = H // GPB                     # heads per core
    DKL = HL * DKv
    NP = DKL // 128
    EC = E // 128
    NPAIR = EC // 2
    NT = S // 128

    nc = _get_nc(S, E, HL, DKv)

    # per-batch host prep (shared by the 4 cores of a batch)
    xb = {}
    for b in range(B):
        xb[b] = {
            "xq": np.ascontiguousarray(
                query[b].T.reshape(EC, 128, 2, S // 2).transpose(2, 1, 0, 3)
                .reshape(2, 128, EC * (S // 2))).astype(BF),
            "xk": np.ascontiguousarray(
                key[b].T.reshape(EC, 128, 2, S // 2).transpose(2, 1, 0, 3)
                .reshape(2, 128, EC * (S // 2))).astype(BF),
            "xv": np.ascontiguousarray(
                value[b].T.reshape(EC, 128, NT, 128).transpose(2, 1, 0, 3)
                .reshape(NT, 128, EC * 128)).astype(BF),
            "mask": np.ascontiguousarray(
                mask[b].reshape(S, NT, 128).transpose(2, 1, 0)).astype(BF),
        }

    in_maps = []
    for c in range(NCORES):
        b, g = c // GPB, c % GPB
        sl = slice(g * DKL, (g + 1) * DKL)
        bqk = np.concatenate([bq[sl].reshape(NP, 128).T,
                              bk[sl].reshape(NP, 128).T], axis=1)
        # wall16: [wq | wk | wv | wo]
        segs = [W[:, sl].reshape(EC, 128, DKL).transpose(1, 0, 2).reshape(
                    128, EC * DKL) for W in (Wq, Wk, Wv)]
        segs.append(Wo[sl, :].reshape(NP, 128, E).transpose(1, 0, 2).reshape(
            128, NP * E))
        wall16 = np.concatenate(segs, axis=1)
        in_maps.append({
            **xb[b],
            "wall16": np.ascontiguousarray(wall16).astype(BF),
            "bqk": np.ascontiguousarray(bqk).astype(np.float32),
            "bv": bv[sl].astype(BF).reshape(1, DKL),
        })

    if _TRACE:
        res = bass_utils.run_bass_kernel_spmd(
            nc, in_maps, core_ids=list(range(NCORES)),
            trace=True, trace_cores=list(_TRACE_CORES))
        _LAST_RESULT = res
        results = res.results
    else:
        results = _get_runner(nc)(in_maps)

    global _LAST_RESULTS_RAW
    _LAST_RESULTS_RAW = results
    y = np.zeros((B, S, E), np.float32)
    for c in range(NCORES):
        y[c // GPB] += results[c]["y"].astype(np.float32)
    y += bo.astype(np.float32)
    return y


def kernel(**inputs):
    return run_sharded(
        inputs["query"], inputs["key"], inputs["value"], inputs["mask"],
        inputs["Wq"], inputs["bq"], inputs["Wk"], inputs["bk"],
        inputs["Wv"], inputs["bv"], inputs["Wo"], inputs["bo"])


# revision 37
# speedup vs baseline: 1.0198x; 1.0198x over previous
"""Multi-head attention (B=2,S=2048,E=1024,H=16,DK=DV=64) on 8 Trainium2 cores.

Sharding: core c handles batch c//4 and head-group c%4 (4 heads each).
Host sums the 4 partial outputs per batch and adds bo.

Design (vs the 281us baseline; all bf16 compute, fp32 psum):
 - The exp stream on the Scalar engine (128 x [128,1024] tiles, ~1.06us
   each = ~136us busy) is the floor; everything else is shaped to keep
   it fed and to minimize ramp + tail.
 - Scores: the two heads of a pair are row-tiled (K=64 each at PE rows
   0-63 / 64-127 via explicit tile_position) and write the two halves
   of ONE [128,1024] fp32 psum tile -> the pair runs concurrently
   (~320ns incl LDW) and one exp call covers both heads (q-half
   granularity, 512 queries x 128 keys x 2 heads per cycle).
 - ctx matmuls are emitted in groups (both q-halves + both heads
   adjacent, same-lhsT back-to-back) so the PE background weight
   buffer hides the LDWEIGHTS; flushed 2+ cycles behind the exp.
 - Mask multiply: one DVE op per cycle with a broadcast AP over both
   head-halves (bf16 2x mode, ~600-690ns).  fp8 anywhere in the
   e/mask path drops DVE to 1x and fp8 projections break the 2e-2
   max-rel budget, so everything stays bf16.
 - Norm per (head, block): den-row copy -> reciprocal_approx_fast
   (psum-sourced reciprocal silently returns garbage; must copy to
   SBUF first) -> gpsimd partition_broadcast -> DVE mul; norms are
   deferred into the next block's early cycles so the PE FIFO never
   blocks on them.  The last block runs qh-outer so its first q-half
   normalizes mid-block and the q0=1024 yproj mostly rides inside it.
 - y casts must be DVE tensor_copy: ScalarE copies truncate fp32->bf16
   and the bias correlates across cores (same rows), blowing max-err.
 - PSUM: st ring 2x2 banks (shared by scores/proj/yproj/keepalive
   tiles) + 2 ctx tiles 2x2 banks = 8.
 - Ramp: split weight DMA + minimal pre-block projections with evacs
   on the then-idle Scalar engine (Identity activation with AP bias).
 - Deferred q/k projection chunks and yproj tiles are spread across
   blocks 0-3 as cycle-inserts sized to each block's PE slack; dummy
   keepalive matmuls bridge block-boundary norm lulls so the PE HAM
   clock stays at 2.4GHz.
"""

import numpy as np
import ml_dtypes

import concourse.bacc as bacc
import concourse.mybir as mybir
import concourse.tile as tile
from concourse import bass_utils

BF = ml_dtypes.bfloat16
F8 = ml_dtypes.float8_e4m3
dt = mybir.dt

NCORES = 8
WSCALE = 1.0
_DEBUG = False


def _emit(nc, tc, inp, y_d, S, E, HL, DK):
    EC = E // 128          # contraction chunks for the projections
    NPAIR = EC // 2        # fp8 DoubleRow chunk pairs
    NT = S // 128          # seq tiles
    DKL = HL * DK          # local head dims (256)
    NP = DKL // 128        # q/k partition tiles (pairs of heads)
    Exp = mybir.ActivationFunctionType.Exp
    DR = mybir.MatmulPerfMode.DoubleRow
    CS = 512
    ESCALE = 0.125 / (WSCALE * WSCALE)
    NQH = S // CS          # 512-wide q-half ranges (4)

    persist = tc.alloc_tile_pool(name="persist", bufs=1)
    qT = [persist.tile([128, S], dt.bfloat16, name=f"qT{m}") for m in range(NP)]
    kT = [persist.tile([128, S], dt.bfloat16, name=f"kT{m}") for m in range(NP)]
    cT = [persist.tile([128, S], dt.bfloat16, name=f"cT{m}") for m in range(NP)]
    VW = HL * (DK + 1)
    vAall = persist.tile([128, NT * VW], dt.bfloat16, name="vAall")
    vA = [vAall[:, t * VW:(t + 1) * VW] for t in range(NT)]

    # bf16 weights: [wq chunks | wk chunks | wv chunks | wo tiles]
    WSEG = EC * DKL
    wall16 = persist.tile([128, 3 * WSEG + NP * E], dt.bfloat16, name="wall16")
    w_sb = {}
    for i, nm in enumerate(("wq", "wk", "wv")):
        w_sb[nm] = [wall16[:, i * WSEG + c * DKL:i * WSEG + (c + 1) * DKL]
                    for c in range(EC)]
    wv_sb = w_sb["wv"]
    wo_sb = [wall16[:, 3 * WSEG + p * E:3 * WSEG + (p + 1) * E]
             for p in range(NP)]
    nc.sync.dma_start(wall16[:, 0:2 * WSEG], inp["wall16"][:, 0:2 * WSEG])
    bqk = persist.tile([128, 2 * NP], dt.float32, name="bqk")
    nc.sync.dma_start(bqk[:], inp["bqk"][:])
    bv1 = persist.tile([1, DKL], dt.bfloat16, name="bv1")
    nc.sync.dma_start(bv1[:], inp["bv"][:])

    # preload the Exp activation-table while the input DMAs stream in
    dumm = persist.tile([1, 4], dt.float32, name="dumm")
    nc.gpsimd.memset(dumm[:], 0.0)
    dumo = persist.tile([1, 4], dt.float32, name="dumo")
    nc.scalar.activation(dumo[:], dumm[:], Exp)

    bvb = persist.tile([128, DKL], dt.bfloat16, name="bvb")
    nc.gpsimd.partition_broadcast(bvb[:], bv1[:])
    nc.gpsimd.memset(vAall[:], 1.0)

    # mask tiles [128, 512] per (qi, t); ring covers 2 blocks + prefetch
    mpool = tc.alloc_tile_pool(name="mask", bufs=1)
    mh = {}
    for qi in range(NQH):
        for t in range(NT):
            mh[(qi, t)] = mpool.tile([128, CS], dt.bfloat16, tag="m",
                                     bufs=32, name=f"mask{qi}_{t}")

    npool = tc.alloc_tile_pool(name="nrm", bufs=1)
    ypool = tc.alloc_tile_pool(name="ysb", bufs=2)
    epool = tc.alloc_tile_pool(name="es", bufs=1)
    ERING = 11

    # bf16 x tiles: one [128, EC*1024] tile per (q/k, seq-half) -> a
    # single DMA with 16KB contiguous per-partition lines
    x8pool = tc.alloc_tile_pool(name="x8", bufs=1)
    xh = {nm: [x8pool.tile([128, EC * (S // 2)], dt.bfloat16, tag="x8",
                           bufs=4, name=f"xh{nm}_{h}") for h in range(2)]
          for nm in ("q", "k")}
    xvpool = tc.alloc_tile_pool(name="xvp", bufs=1)
    xvt = [xvpool.tile([128, EC * 128], dt.bfloat16, tag="xv", bufs=10,
                       name=f"xvt{t}") for t in range(NT)]

    # ---- DMA schedule (issue order ~= completion order) -------------------
    def dma_x8(nm, h):
        nc.sync.dma_start(xh[nm][h][:], inp["x" + nm][h])

    dma_x8("q", 0)
    dma_x8("k", 0)
    for t in range(4):
        nc.sync.dma_start(mh[(0, t)][:], inp["mask"][:, t, 0:CS])
    dma_x8("k", 1)
    nc.sync.dma_start(wall16[:, 2 * WSEG:], inp["wall16"][:, 2 * WSEG:])
    for tv in range(NT):
        nc.sync.dma_start(xvt[tv][:], inp["xv"][tv])
        if tv < 12:
            for t in range(4 + 2 * tv, 4 + 2 * (tv + 1)):
                qi, tt = divmod(t, NT)
                if qi < 2:
                    nc.sync.dma_start(mh[(qi, tt)][:],
                                      inp["mask"][:, tt, qi * CS:(qi + 1) * CS])
    dma_x8("q", 1)
    for t in range(28, 2 * NT):
        qi, tt = divmod(t, NT)
        nc.sync.dma_start(mh[(qi, tt)][:],
                          inp["mask"][:, tt, qi * CS:(qi + 1) * CS])
    for qi in (2, 3):
        for tt in range(NT):
            nc.sync.dma_start(mh[(qi, tt)][:],
                              inp["mask"][:, tt, qi * CS:(qi + 1) * CS])

    stpool = tc.alloc_tile_pool(name="stps", bufs=1, space="PSUM")

    PROJ = {"q": (qT, 0), "k": (kT, NP)}
    _kan = [0]

    def _proj_group(nm, m, col, act_evac=False):
        # one 512-col range of the q or k projection (bf16)
        dst, boff = PROJ[nm]
        h, n0 = divmod(col, S // 2)
        ps = stpool.tile([128, CS], dt.float32, tag="st", bufs=2,
                         name=f"{nm}ps{m}_{col}")
        for c in range(EC):
            nc.tensor.matmul(ps[:], w_sb["w" + nm][c][:, 128 * m:128 * (m + 1)],
                             xh[nm][h][:, c * (S // 2) + n0:
                                        c * (S // 2) + n0 + CS],
                             start=(c == 0), stop=(c == EC - 1))
        if act_evac:
            # ramp-time evac on the (idle) Scalar engine
            nc.scalar.activation(dst[m][:, col:col + CS], ps[:],
                                 mybir.ActivationFunctionType.Identity,
                                 bias=bqk[:, boff + m:boff + m + 1])
        else:
            nc.vector.tensor_scalar_add(
                dst[m][:, col:col + CS], ps[:],
                bqk[:, boff + m:boff + m + 1])

    def _v_group(t):
        vps = stpool.tile([128, CS], dt.float32, tag="st", bufs=2,
                          name=f"vps{t}")
        for c in range(EC):
            nc.tensor.matmul(vps[:, 0:DKL],
                             xvt[t][:, c * 128:(c + 1) * 128],
                             wv_sb[c][:],
                             start=(c == 0), stop=(c == EC - 1))
        nc.vector.tensor_add(
            vA[t][:].rearrange("p (h c) -> p h c", h=HL)[:, :, 0:DK],
            vps[:, 0:DKL].rearrange("p (h c) -> p h c", h=HL),
            bvb[:].rearrange("p (h c) -> p h c", h=HL))

    def _keepalive(n):
        # dummy matmuls into a throwaway st-ring slot: keep the PE HAM-warm
        # across block-boundary norm lulls (idle >3.4us drops PE to 1.2GHz)
        ka = stpool.tile([128, CS], dt.float32, tag="st", bufs=2,
                         name=f"ka{_kan[0]}")
        _kan[0] += 1
        for i in range(n):
            nc.tensor.matmul(ka[:, 0:DKL], bvb[:, 0:128], bvb[:],
                             start=True, stop=True)

    def _yproj_group(s, tail=False):
        yp = stpool.tile([128, E], dt.float32, tag="st", bufs=2,
                         name=f"yp{s}")
        for p in range(NP):
            for e0 in range(0, E, CS):
                nc.tensor.matmul(yp[:, e0:e0 + CS],
                                 cT[p][:, s * 128:(s + 1) * 128],
                                 wo_sb[p][:, e0:e0 + CS],
                                 start=(p == 0), stop=(p == NP - 1))
        ysb = ypool.tile([128, E], dt.bfloat16, tag="y", bufs=2,
                         name=f"ysb{s}")
        nc.vector.tensor_copy(ysb[:], yp[:])
        nc.sync.dma_start(y_d[s * 128:(s + 1) * 128, :], ysb[:])

    def _norm(ctx_t, hp, h, q0, w=None):
        # normalize cT[hp] rows sub..sub+64: dn copy -> recip ->
        # gpsimd bcast -> mul.  w=None: full 1024 cols; w=0/1: one 512 half.
        sub = (h % 2) * 64
        wd = 2 * CS if w is None else CS
        col = q0 + (0 if w is None else w * CS)
        dn = npool.tile([1, 2 * CS], dt.float32, tag="dn",
                        name=f"dn{h}_{col}")
        nc.vector.tensor_copy(dn[:, 0:wd], ctx_t[DK:DK + 1,
                                                 (col - q0):(col - q0) + wd])
        rec32 = npool.tile([1, 2 * CS], dt.float32, tag="rec32",
                           name=f"rec32_{h}_{col}")
        nc.vector.reciprocal_approx_fast(rec32[:, 0:wd], dn[:, 0:wd])
        if _DEBUG and h == 0 and col == 0 and w is None:
            nc.sync.dma_start(inp["drec"][:], rec32[:])
            nc.sync.dma_start(inp["dden"][:], dn[:])
        bd = npool.tile([64, 2 * CS], dt.float32, tag="bd", bufs=2,
                        name=f"bd{h}_{col}")
        nc.gpsimd.partition_broadcast(bd[:, 0:wd], rec32[:, 0:wd], opt=False)
        nc.vector.tensor_mul(cT[hp][sub:sub + 64, col:col + wd],
                             ctx_t[0:DK, (col - q0):(col - q0) + wd],
                             bd[:, 0:wd])

    # ---- pre-block projections ---------------------------------------------
    _proj_group("q", 0, 0, act_evac=True)
    _proj_group("k", 0, 0, act_evac=True)

    # ---- attention blocks --------------------------------------------------
    # blocks: (q0=0,hp0), (q0=0,hp1), (q0=1024,hp0), (q0=1024,hp1)
    # inserts are (cycle -> thunk) maps; cycles 0..31.  The last block runs
    # qh-outer so its first q-half can normalize + start yproj mid-block.
    with tc.tile_pool(name="ctxps", bufs=1, space="PSUM") as ctxpool:
        blocks = [(0, 0), (0, 1), (1024, 0), (1024, 1)]
        pending_norms = []
        for bi, (q0, hp) in enumerate(blocks):
            heads = (2 * hp, 2 * hp + 1)
            ctxs = {}
            pend = []
            inserts = {}

            def ins(cyc, fn):
                inserts.setdefault(cyc, []).append(fn)

            for i, fn in enumerate(pending_norms):
                ins(1 + 2 * i, fn)
            pending_norms = []

            if bi == 0:
                ins(0, lambda: _proj_group("k", 0, CS))
                ins(10, lambda: _proj_group("q", 0, CS))
                for i in range(2):
                    ins(4 + 2 * i, lambda i=i: _proj_group("k", 0, 1024 + CS * i))
                for i in range(16):
                    ins(8 + i, lambda i=i: _v_group(i))
                ins(22, lambda: _proj_group("k", 1, 0))
                ins(24, lambda: _proj_group("k", 1, CS))
                ins(26, lambda: _proj_group("q", 1, 0))
                ins(28, lambda: _proj_group("q", 1, CS))
            elif bi == 1:
                for i in range(2):
                    ins(1 + 2 * i, lambda i=i: _proj_group("k", 1, 1024 + CS * i))
                for i in range(2):
                    ins(5 + 2 * i, lambda i=i: _proj_group("q", 0, 1024 + CS * i))
            elif bi == 2:
                for i in range(2):
                    ins(2 + 2 * i, lambda i=i: _proj_group("q", 1, 1024 + CS * i))
                for i in range(6):
                    ins(6 + 3 * i, lambda i=i: _yproj_group(i))
            elif bi == 3:
                # qh-outer: qa complete at cyc 15; norms + q0=0 leftovers +
                # q0=1024 first-half yproj ride the qb cycles
                ins(5, lambda: _yproj_group(6))
                ins(7, lambda: _yproj_group(7))
                ins(18, lambda: _norm(ctxs[heads[0]], hp, heads[0], q0, w=0))
                ins(20, lambda: _norm(ctxs[heads[1]], hp, heads[1], q0, w=0))
                for i in range(4):
                    ins(22 + 2 * i, lambda i=i: _yproj_group(8 + i))

            def _ctx_emit_group(entries):
                # grouped by head so same-lhsT matmuls are adjacent (the
                # PE background weight buffer then hides the LDWEIGHTS)
                for ci, h in enumerate(heads):
                    if h not in ctxs:
                        ctxs[h] = ctxpool.tile(
                            [128, 2 * CS], dt.float32,
                            tag=f"ctx{h % 2}", name=f"ctx{h}_{q0}")
                    for (t_, qh_, e_) in entries:
                        nc.tensor.matmul(
                            ctxs[h][0:DK + 1, qh_ * CS:(qh_ + 1) * CS],
                            vA[t_][:, h * (DK + 1):(h + 1) * (DK + 1)],
                            e_[:, ci * CS:(ci + 1) * CS],
                            start=(t_ == 0), stop=(t_ == NT - 1))
                for p in entries:
                    pend.remove(p)

            def _ctx_flush(upto, limit=None):
                ready = [p for p in pend if p[0] <= upto]
                if limit is not None:
                    ready = ready[:limit]
                if ready:
                    _ctx_emit_group(ready)

            if bi in (0, 3):
                order = [(t, qh) for qh in range(2) for t in range(NT)]
            else:
                order = [(t, qh) for t in range(NT) for qh in range(2)]
            for cyc, (t, qh) in enumerate(order):
                qc = q0 + qh * CS
                qi = qc // CS
                st = stpool.tile([128, 2 * CS], dt.float32, tag="st",
                                 bufs=2, name=f"st{t}_{qh}_{bi}")
                for ci, h in enumerate(heads):
                    sub = (h % 2) * 64
                    nc.tensor.matmul(
                        st[:, ci * CS:(ci + 1) * CS],
                        kT[hp][sub:sub + DK, t * 128:(t + 1) * 128],
                        qT[hp][sub:sub + DK, qc:qc + CS],
                        start=True, stop=True,
                        tile_position=(sub, 0))
                e = epool.tile([128, 2 * CS], dt.bfloat16, tag="e",
                               bufs=ERING, name=f"e{t}_{qh}_{bi}")
                nc.scalar.activation(e[:], st[:], Exp, scale=ESCALE)
                nc.vector.tensor_mul(
                    e[:].rearrange("p (two n) -> p two n", two=2),
                    e[:].rearrange("p (two n) -> p two n", two=2),
                    mh[(qi, t)][:].unsqueeze(1).broadcast_to(
                        [128, 2, CS]))
                if _DEBUG and bi == 0 and t == 0 and qh == 0:
                    nc.sync.dma_start(inp["de0"][:], e[:])
                pend.append((t, qh, e))
                for fn in inserts.get(cyc, ()):
                    if fn is not None:
                        fn()
                if cyc % 2 == 0:
                    continue
                if bi == 0:
                    grp = [p for p in pend[:-2] if 9 + p[0] <= cyc][:6]
                    if grp:
                        _ctx_emit_group(grp)
                elif bi == 3:
                    # keep a 2-cycle lag behind production order
                    grp = list(pend[:-2])[:6]
                    if grp:
                        _ctx_emit_group(grp)
                else:
                    _ctx_flush(t - 2, limit=6)
            _ctx_flush(NT - 1)
            if bi < 3:
                _keepalive(14)
            if bi == 3:
                _norm(ctxs[heads[0]], hp, heads[0], q0, w=1)
                _norm(ctxs[heads[1]], hp, heads[1], q0, w=1)
                _keepalive(24)
            else:
                pending_norms = [
                    (lambda h=h, c=ctxs, hp=hp, q0=q0: _norm(c[h], hp, h, q0))
                    for h in heads]
        # tail: last yproj quartet
        for s in range(12, 16):
            _yproj_group(s)

    if _DEBUG:
        for nm, tl in [("dqT0", qT[0]), ("dqT1", qT[1]), ("dkT0", kT[0]),
                       ("dkT1", kT[1]), ("dcT0", cT[0]), ("dcT1", cT[1])]:
            nc.sync.dma_start(inp[nm][:], tl[:])
        nc.sync.dma_start(inp["dvA"][:], vAall[:])

    xvpool.release()
    x8pool.release()
    epool.release()
    ypool.release()
    npool.release()
    stpool.release()
    mpool.release()
    persist.release()


def _build(S, E, HL, DK):
    EC = E // 128
    NPAIR = EC // 2
    NT = S // 128
    DKL = HL * DK
    NP = DKL // 128
    nc = bacc.Bacc("TRN2", target_bir_lowering=False, debug=False,
                   num_devices=NCORES)
    inp = {}
    for nm in ("xq", "xk"):
        inp[nm] = nc.dram_tensor(nm, [2, 128, EC * (S // 2)], dt.bfloat16,
                                 kind="ExternalInput").ap()
    inp["xv"] = nc.dram_tensor("xv", [NT, 128, EC * 128], dt.bfloat16,
                               kind="ExternalInput").ap()
    inp["wall16"] = nc.dram_tensor("wall16", [128, 3 * EC * DKL + NP * E],
                                   dt.bfloat16, kind="ExternalInput").ap()
    inp["bqk"] = nc.dram_tensor("bqk", [128, 2 * NP], dt.float32,
                                kind="ExternalInput").ap()
    inp["bv"] = nc.dram_tensor("bv", [1, DKL], dt.bfloat16,
                               kind="ExternalInput").ap()
    inp["mask"] = nc.dram_tensor("mask", [128, NT, S], dt.bfloat16,
                                 kind="ExternalInput").ap()
    if _DEBUG:
        for nm in ("dqT0", "dqT1", "dkT0", "dkT1", "dcT0", "dcT1"):
            inp[nm] = nc.dram_tensor(nm, [128, S], dt.bfloat16,
                                     kind="ExternalOutput").ap()
        inp["dvA"] = nc.dram_tensor("dvA", [128, NT * HL * (DK + 1)],
                                    dt.bfloat16, kind="ExternalOutput").ap()
        inp["de0"] = nc.dram_tensor("de0", [128, 1024], dt.bfloat16,
                                    kind="ExternalOutput").ap()
        inp["drec"] = nc.dram_tensor("drec", [1, 1024], dt.float32,
                                     kind="ExternalOutput").ap()
        inp["dden"] = nc.dram_tensor("dden", [1, 1024], dt.float32,
                                     kind="ExternalOutput").ap()
    y_d = nc.dram_tensor("y", [S, E], dt.bfloat16, kind="ExternalOutput").ap()

    with tile.TileContext(nc) as tc:
        _emit(nc, tc, inp, y_d, S, E, HL, DK)
    nc.compile()
    return nc


_CACHE = {}
_TRACE = False
_TRACE_CORES = (0,)
_LAST_RESULT = None


def _get_nc(S, E, HL, DK):
    key = (S, E, HL, DK)
    if key not in _CACHE:
        _CACHE[key] = _build(S, E, HL, DK)
    return _CACHE[key]


_RUNNER_CACHE = {}


def _get_runner(nc):
    """Cached variant of bass2jax.run_bass_via_pjrt's multi-core path: build
    the jitted shard_map executable once and reuse it across kernel() calls
    (a fresh jax.jit per call re-traces and may recompile)."""
    if id(nc) in _RUNNER_CACHE:
        return _RUNNER_CACHE[id(nc)]
    import jax
    import concourse.mybir as _mybir
    from concourse import bass2jax
    from jax.sharding import Mesh, PartitionSpec
    from jax.experimental.shard_map import shard_map

    bass2jax.install_neuronx_cc_hook()
    pid_name = nc.partition_id_tensor.name if nc.partition_id_tensor else None
    in_names, out_names, out_avals, zero_shapes = [], [], [], []
    for alloc in nc.m.functions[0].allocations:
        if not isinstance(alloc, _mybir.MemoryLocationSet):
            continue
        name = alloc.memorylocations[0].name
        if alloc.kind == "ExternalInput":
            if name != pid_name:
                in_names.append(name)
        elif alloc.kind == "ExternalOutput":
            out_names.append(name)
            shape = tuple(alloc.tensor_shape)
            dtype = _mybir.dt.np(alloc.dtype)
            out_avals.append(jax.core.ShapedArray(shape, dtype))
            zero_shapes.append((shape, dtype))
    n_params = len(in_names)
    n_outs = len(out_avals)
    all_names = in_names + out_names
    if pid_name is not None:
        all_names = all_names + [pid_name]

    def _body(*args):
        operands = list(args)
        if pid_name is not None:
            operands.append(bass2jax.partition_id_tensor())
        return tuple(bass2jax._bass_exec_p.bind(
            *operands,
            out_avals=tuple(out_avals),
            in_names=tuple(all_names),
            out_names=tuple(out_names),
            lowering_input_output_aliases=(),
            sim_require_finite=True,
            sim_require_nnan=True,
            nc=nc,
        ))

    devices = jax.devices()[:NCORES]
    mesh = Mesh(np.asarray(devices), ("core",))
    donate = tuple(range(n_params, n_params + n_outs))
    sharded = jax.jit(
        shard_map(_body, mesh=mesh,
                  in_specs=(PartitionSpec("core"),) * (n_params + n_outs),
                  out_specs=(PartitionSpec("core"),) * n_outs,
                  check_rep=False),
        donate_argnums=donate, keep_unused=True)

    def run(in_maps):
        concat_in = [np.concatenate([np.asarray(m[nm]) for m in in_maps], axis=0)
                     for nm in in_names]
        concat_zeros = [np.zeros((NCORES * s[0], *s[1:]), d)
                        for s, d in zero_shapes]
        outs = sharded(*concat_in, *concat_zeros)
        return [
            {nm: np.asarray(outs[i]).reshape(NCORES, *out_avals[i].shape)[c]
             for i, nm in enumerate(out_names)}
            for c in range(NCORES)
        ]

    _RUNNER_CACHE[id(nc)] = run
    return run


def run_sharded(query, key, value, mask, Wq, bq, Wk, bk, Wv, bv, Wo, bo):
    """Full-input -> full-output runner (generic shapes)."""
    global _LAST_RESULT
    query, key, value = (np.asarray(a, np.float32) for a in (query, key, value))
    mask = np.asarray(mask)
    Wq, bq, Wk, bk, Wv, bv, Wo, bo = (
        np.asarray(a, np.float32) for a in (Wq, bq, Wk, bk, Wv, bv, Wo, bo))

    B, S, E = query.shape
    HDK = Wq.shape[1]
    DKv = 64
    H = HDK // DKv
    GPB = NCORES // B                 # cores per batch
    HL = H // GPB                     # heads per core
    DKL = HL * DKv
    NP = DKL // 128
    EC = E // 128
    NPAIR = EC // 2
    NT = S // 128

    nc = _get_nc(S, E, HL, DKv)

    # per-batch host prep (shared by the 4 cores of a batch)
    xb = {}
    for b in range(B):
        xb[b] = {
            "xq": np.ascontiguousarray(
                query[b].T.reshape(EC, 128, 2, S // 2).transpose(2, 1, 0, 3)
                .reshape(2, 128, EC * (S // 2))).astype(BF),
            "xk": np.ascontiguousarray(
                key[b].T.reshape(EC, 128, 2, S // 2).transpose(2, 1, 0, 3)
                .reshape(2, 128, EC * (S // 2))).astype(BF),
            "xv": np.ascontiguousarray(
                value[b].T.reshape(EC, 128, NT, 128).transpose(2, 1, 0, 3)
                .reshape(NT, 128, EC * 128)).astype(BF),
            "mask": np.ascontiguousarray(
                mask[b].reshape(S, NT, 128).transpose(2, 1, 0)).astype(BF),
        }

    in_maps = []
    for c in range(NCORES):
        b, g = c // GPB, c % GPB
        sl = slice(g * DKL, (g + 1) * DKL)
        bqk = np.concatenate([bq[sl].reshape(NP, 128).T,
                              bk[sl].reshape(NP, 128).T], axis=1)
        # wall16: [wq | wk | wv | wo]
        segs = [W[:, sl].reshape(EC, 128, DKL).transpose(1, 0, 2).reshape(
                    128, EC * DKL) for W in (Wq, Wk, Wv)]
        segs.append(Wo[sl, :].reshape(NP, 128, E).transpose(1, 0, 2).reshape(
            128, NP * E))
        wall16 = np.concatenate(segs, axis=1)
        in_maps.append({
            **xb[b],
            "wall16": np.ascontiguousarray(wall16).astype(BF),
            "bqk": np.ascontiguousarray(bqk).astype(np.float32),
            "bv": bv[sl].astype(BF).reshape(1, DKL),
        })

    if _TRACE:
        res = bass_utils.run_bass_kernel_spmd(
            nc, in_maps, core_ids=list(range(NCORES)),
            trace=True, trace_cores=list(_TRACE_CORES))
        _LAST_RESULT = res
        results = res.results
    else:
        results = _get_runner(nc)(in_maps)

    global _LAST_RESULTS_RAW
    _LAST_RESULTS_RAW = results
    y = np.zeros((B, S, E), np.float32)
    for c in range(NCORES):
        y[c // GPB] += results[c]["y"].astype(np.float32)
    y += bo.astype(np.float32)
    return y


def kernel(**inputs):
    return run_sharded(
        inputs["query"], inputs["key"], inputs["value"], inputs["mask"],
        inputs["Wq"], inputs["bq"], inputs["Wk"], inputs["bk"],
        inputs["Wv"], inputs["bv"], inputs["Wo"], inputs["bo"])
